# revision 65
# baseline (speedup 1.0000x reference)
"""Trainium2 Bass kernel v2: conv-projected MHA, engine-parallel softmax.

Per batch item (one NeuronCore each):
  y = BN(depthwise3x3(x))  ->  q/k/v proj  ->  attention  ->  out proj

Key performance structure vs v1:
 - softmax exp split across ACT (exact table exp) and DVE (Schraudolph
   bit-trick exp via tensor_scalar f32->int16, bitcast to bf16); logits
   span only ~[-3.1, 3.1] so the 3% approx error yields ~1e-2 final
   rel err (tolerance 2e-2).
 - scores: 4-way row-tiled (tile_position (32j,0)) K=32 matmuls into two
   [128,1024] PSUM tiles from a 3-slot ring -> PE/ACT/DVE fully pipelined.
 - PV: quad column-tiled per head pair: v (M=32) at (0,0)/(0,64) and the
   softmax-denominator ones-column (M=1) at (0,32)/(0,96), bf16.
 - renorm: gather D rows {32,96} to partitions 0/1 via indicator matmul,
   one batched reciprocal, K=2 broadcast matmul, tensor_tensor multiply.
 - PSUM: "s" tag 3x2 banks + "ov" 1 + "sgrb" 1 = 8 banks.
 - software pipeline: PV/renorm of iteration i-1 interleaved into the
   scores/exp stream of iteration i.
"""
import sys

sys.path.insert(0, "/opt/trn_rl_repo")
from contextlib import ExitStack

import numpy as np
import ml_dtypes

BF16NP = ml_dtypes.bfloat16

B, T, C = 8, 1024, 384
NH, DH = 12, 32
HH = WW = 32
SCALE = float(DH) ** 0.5
BN_EPS = 1e-5
NCORES = 8

# Schraudolph exp constants (bf16 / int16 variant), x = logits
A16 = 2.0 ** 7 / np.log(2.0)  # 184.664965...
B16 = 16250.0
A16S = A16 * SCALE  # folded logit scale (psum holds q.k without scale)

# which B-tiles (heads 2,3 of each group) go to DVE per (lh,g); rest ACT
DVE_B_TTS = (0, 1, 2, 3, 4, 5)

_CACHE = {}


def _build(debug=False):
    import concourse.bass as bass
    import concourse.tile as tile
    from concourse import bacc, mybir
    from concourse.masks import make_identity

    F32 = mybir.dt.float32
    F32R = mybir.dt.float32r
    BF16 = mybir.dt.bfloat16
    I16 = mybir.dt.int16
    AF = mybir.ActivationFunctionType
    ALU = mybir.AluOpType

    nc = bacc.Bacc("TRN2", target_bir_lowering=False, debug=False)

    xpad_d = nc.dram_tensor("xpad", [C, 34 * 34], BF16, kind="ExternalInput").ap()
    diag_d = nc.dram_tensor("diag", [3, 128, 9 * 128], BF16, kind="ExternalInput").ap()
    bias_d = nc.dram_tensor("bias", [C, 1], F32, kind="ExternalInput").ap()
    wqT_d = nc.dram_tensor("wqT", [C, C], F32R, kind="ExternalInput").ap()
    wkT_d = nc.dram_tensor("wkT", [C, C], F32R, kind="ExternalInput").ap()
    wvT_d = nc.dram_tensor("wvT", [C, C], F32R, kind="ExternalInput").ap()
    woP_d = nc.dram_tensor("woP", [6, 128, C], F32R, kind="ExternalInput").ap()
    indg_d = nc.dram_tensor("indg", [128, 2], F32R, kind="ExternalInput").ap()
    bind2_d = nc.dram_tensor("bind2", [2, 128], F32R, kind="ExternalInput").ap()
    outT_d = nc.dram_tensor("outT", [C, T], BF16, kind="ExternalOutput").ap()
    dbg = {}
    if debug:
        dbg["y"] = nc.dram_tensor("dbg_y", [C, T], F32, kind="ExternalOutput").ap()
        dbg["qT"] = nc.dram_tensor("dbg_qT", [C, T], F32, kind="ExternalOutput").ap()
        dbg["E"] = nc.dram_tensor(
            "dbg_E", [2, 2, T, 1024], BF16, kind="ExternalOutput"
        ).ap()  # (lh=0,g=0) A/B tiles
        dbg["ovs"] = nc.dram_tensor(
            "dbg_ovs", [2, 128, 512], F32, kind="ExternalOutput"
        ).ap()  # (lh0,g0) pairs
        dbg["attn"] = nc.dram_tensor(
            "dbg_attn", [6, 128, T], F32, kind="ExternalOutput"
        ).ap()

    CT = C // 128  # 3 c-tiles / head groups
    TT = T // 128  # 8 t-tiles
    TH = T // 512  # 2 l halves

    with tile.TileContext(nc) as tc, ExitStack() as top:
        persist = top.enter_context(tc.tile_pool(name="persist", bufs=1))
        sps = top.enter_context(tc.tile_pool(name="sps", bufs=3, space="PSUM"))
        auxps = top.enter_context(tc.tile_pool(name="auxps", bufs=1, space="PSUM"))
        epool = top.enter_context(tc.tile_pool(name="epool", bufs=26))
        ovspool = top.enter_context(tc.tile_pool(name="ovspool", bufs=4))
        rrpool = top.enter_context(tc.tile_pool(name="rrpool", bufs=3))
        ocpool = top.enter_context(tc.tile_pool(name="ocpool", bufs=4))

        y_sb = [persist.tile([128, T], F32R, tag=f"y{i}", name=f"y{i}") for i in range(CT)]
        qT_sb = [persist.tile([128, T], F32R, tag=f"q{i}", name=f"q{i}") for i in range(CT)]
        kT_sb = [persist.tile([128, T], F32R, tag=f"k{i}", name=f"k{i}") for i in range(CT)]
        vaug = [persist.tile([128, NH, 34], BF16, tag=f"va{i}", name=f"va{i}") for i in range(TT)]
        attn = [persist.tile([128, T], F32R, tag=f"at{i}", name=f"at{i}") for i in range(6)]
        bias_sb = [persist.tile([128, 1], F32, tag=f"b{i}", name=f"b{i}") for i in range(CT)]
        indg_sb = persist.tile([128, 2], F32R, tag="indg")
        bind2_sb = persist.tile([2, 128], F32R, tag="bind2")
        wT_sb = {}
        woP_sb = [persist.tile([128, C], F32R, tag=f"woP{i}", name=f"woP{i}") for i in range(6)]

        # ---------------- prefix: conv -> y ----------------
        with ExitStack() as ph1:
            convpool = ph1.enter_context(tc.tile_pool(name="convpool", bufs=1))
            xp = [convpool.tile([128, 34 * 34], BF16, tag=f"xp{i}", name=f"xp{i}") for i in range(CT)]
            diag = [convpool.tile([128, 9, 128], BF16, tag=f"dg{i}", name=f"dg{i}") for i in range(CT)]

            # PE p-state warmup on a dedicated uninitialized SBUF tile
            # (no input deps): burns the clock-ramp window before inputs land
            warm_sb = convpool.tile([128, 512], BF16, tag="warm")
            nc.vector.memset(warm_sb[:].bitcast(F32), 1.0)
            for tt in range(TT):
                nc.vector.memset(vaug[tt][:, :, 32:34].bitcast(F32), 0.0)
                nc.vector.memset(vaug[tt][:, :, 32:33], 1.0)
            for wi in range(10):
                wp = sps.tile([128, 512], F32, tag="s", name="warm")
                nc.tensor.matmul(
                    wp[:], warm_sb[:, 0:128], warm_sb[:],
                    start=True, stop=True, skip_group_check=True,
                )
            for nm in ("q", "k", "v"):
                wT_sb[nm] = [persist.tile([128, C], F32R, tag=f"w{nm}{i}", name=f"w{nm}{i}") for i in range(CT)]
            for i in (0, 1):
                nc.sync.dma_start(xp[i][:], xpad_d[i * 128 : (i + 1) * 128, :])
                nc.sync.dma_start(
                    diag[i][:].rearrange("p a b -> p (a b)"), diag_d[i]
                )
                nc.sync.dma_start(bias_sb[i][:], bias_d[i * 128 : (i + 1) * 128, :])
            for nm, d in (("q", wqT_d), ("k", wkT_d)):
                for i in range(CT):
                    nc.sync.dma_start(wT_sb[nm][i][:], d[i * 128 : (i + 1) * 128, :])
            nc.sync.dma_start(xp[2][:], xpad_d[256:384, :])
            nc.sync.dma_start(diag[2][:].rearrange("p a b -> p (a b)"), diag_d[2])
            nc.sync.dma_start(bias_sb[2][:], bias_d[256:384, :])
            for i in range(CT):
                nc.sync.dma_start(wT_sb["v"][i][:], wvT_d[i * 128 : (i + 1) * 128, :])
            for i in range(6):
                nc.sync.dma_start(woP_sb[i][:], woP_d[i])
            nc.sync.dma_start(indg_sb[:], indg_d)
            nc.sync.dma_start(bind2_sb[:], bind2_d)

            # conv: 9 accumulating diag matmuls per (c-tile, t-half)
            for i in range(CT):
                for th in range(TH):
                    yp = sps.tile([128, 512], F32, tag="s", name="convps")
                    r0 = th * 16
                    for k in range(9):
                        dy, dx = k // 3 - 1, k % 3 - 1
                        off = (r0 + 1 + dy) * 34 + (1 + dx)
                        rhs = bass.AP(
                            tensor=xp[i].tensor,
                            offset=xp[i].offset + off,
                            ap=[list(p) for p in xp[i].ap[:1]] + [[34, 16], [1, 32]],
                        )
                        nc.tensor.matmul(
                            yp[:].rearrange("p (a b) -> p a b", a=16),
                            diag[i][:, k, :],
                            rhs,
                            start=(k == 0),
                            stop=(k == 8),
                        )
                    nc.vector.tensor_scalar_add(
                        y_sb[i][:, th * 512 : (th + 1) * 512], yp[:], bias_sb[i][:]
                    )
            if debug:
                for i in range(CT):
                    nc.sync.dma_start(
                        dbg["y"][i * 128 : (i + 1) * 128, :], y_sb[i][:].bitcast(F32)
                    )

        # ---------------- prefix: q/k group 0 ----------------
        def emit_qk(ot):
            for nm, dst in (("q", qT_sb), ("k", kT_sb)):
                for th in range(TH):
                    pp = sps.tile([128, 512], F32, tag="s", name="qkps")
                    for kt in range(CT):
                        nc.tensor.matmul(
                            pp[:],
                            wT_sb[nm][kt][:, ot * 128 : (ot + 1) * 128],
                            y_sb[kt][:, th * 512 : (th + 1) * 512],
                            start=(kt == 0),
                            stop=(kt == CT - 1),
                        )
                    nc.scalar.activation(
                        dst[ot][:, th * 512 : (th + 1) * 512], pp[:], AF.Copy
                    )

        def emit_v(tts):
            for tt in tts:
                vp = sps.tile([128, C], F32, tag="s", name="vps")
                for kt in range(CT):
                    nc.tensor.matmul(
                        vp[:],
                        y_sb[kt][:, tt * 128 : (tt + 1) * 128],
                        wT_sb["v"][kt][:],
                        start=(kt == 0),
                        stop=(kt == CT - 1),
                    )
                nc.vector.tensor_copy(
                    vaug[tt][:, :, 0:32], vp[:].rearrange("p (h d) -> p h d", h=NH)
                )

        emit_qk(0)
        if debug:
            pass  # qT dumped at end

        # ---------------- attention ----------------
        LHG = [(lh, g) for lh in range(TH) for g in range(CT)]

        def emit_pv(st, p):
            g = st["g"]
            ov = auxps.tile([128, 512], F32, tag="ov", name="ov", bufs=2)
            E = st["eA"] if p == 0 else st["eB"]
            h_e = 4 * g + 2 * p
            h_o = h_e + 1
            for tt in range(TT):
                s0, s1 = (tt == 0), (tt == TT - 1)
                nc.tensor.matmul(
                    ov[0:33, :], vaug[tt][:, h_e, 0:33], E[tt][:, 0:512],
                    start=s0, stop=s1, tile_position=(0, 0),
                )
                nc.tensor.matmul(
                    ov[64:97, :], vaug[tt][:, h_o, 0:33], E[tt][:, 512:1024],
                    start=s0, stop=s1, tile_position=(0, 64),
                )
            st["ov"][p] = ov

        def emit_gather(st, p):
            ovs = ovspool.tile([128, 512], F32R, tag="ovs", name="ovs")
            nc.vector.tensor_copy(ovs[:], st["ov"][p][:])
            st["ovs"][p] = ovs
            sg = sps.tile([4, 512], F32, tag="s", name="sg")
            nc.tensor.matmul(
                sg[0:2, :], indg_sb[:], st["ovs"][p][:], start=True, stop=True
            )
            st["sg"][p] = sg

        def emit_finish(st, p):
            lh, g = st["lh"], st["g"]
            rrf = rrpool.tile([2, 512], F32, tag="rrf", name="rrf")
            nc.vector.reciprocal_approx_fast(rrf[0:2, :], st["sg"][p][0:2, :])
            rr = rrpool.tile([2, 512], F32R, tag="rr", name="rr")
            nc.scalar.activation(rr[0:2, :], rrf[0:2, :], AF.Copy)
            Rb = sps.tile([128, 512], F32, tag="s", name="Rb")
            nc.tensor.matmul(
                Rb[:], bind2_sb[:], rr[0:2, :], start=True, stop=True
            )
            at = attn[2 * g + p]
            nc.vector.tensor_tensor(
                at[:, lh * 512 : (lh + 1) * 512], st["ovs"][p][:].bitcast(F32),
                Rb[:], ALU.mult,
            )
            if debug and lh == 0 and g == 0:
                nc.sync.dma_start(dbg["ovs"][p], st["ovs"][p][:].bitcast(F32))

        op_done = set()

        def emit_op(ot, th, ceng):
            op = sps.tile([128, 512], F32, tag="s", name="op")
            for pt in range(6):
                nc.tensor.matmul(
                    op[:],
                    woP_sb[pt][:, ot * 128 : (ot + 1) * 128],
                    attn[pt][:, th * 512 : (th + 1) * 512],
                    start=(pt == 0),
                    stop=(pt == 5),
                )
            oc = ocpool.tile([128, 512], BF16, tag="oc", name="oc")
            nc.scalar.activation(oc[:], op[:], AF.Copy)
            nc.sync.dma_start(
                outT_d[ot * 128 : (ot + 1) * 128, th * 512 : (th + 1) * 512],
                oc[:],
            )
            op_done.add((ot, th))

        prev = None
        for it, (lh, g) in enumerate(LHG):
            cur = {
                "lh": lh, "g": g, "eA": [], "eB": [],
                "ov": [None, None], "ovs": [None, None], "sg": [None, None],
            }
            for tt in range(TT):
                sA = sps.tile([128, 1024], F32, tag="s", name="sA")
                sB = sps.tile([128, 1024], F32, tag="s", name="sB")
                for j, dst in (
                    (0, sA[:, 0:512]),
                    (1, sA[:, 512:1024]),
                    (2, sB[:, 0:512]),
                    (3, sB[:, 512:1024]),
                ):
                    nc.tensor.matmul(
                        dst,
                        kT_sb[g][32 * j : 32 * (j + 1), tt * 128 : (tt + 1) * 128],
                        qT_sb[g][32 * j : 32 * (j + 1), lh * 512 : (lh + 1) * 512],
                        start=True,
                        stop=True,
                        tile_position=(32 * j, 0),
                    )
                eA = epool.tile([128, 1024], BF16, tag="E", name="eA")
                eB = epool.tile([128, 1024], BF16, tag="E", name="eB")
                nc.scalar.activation(eA[:], sA[:], AF.Exp, scale=SCALE)
                if tt in DVE_B_TTS or prev is None:
                    nc.vector.tensor_scalar(
                        eB[:].bitcast(I16), sB[:], float(A16S), float(B16),
                        ALU.mult, ALU.add,
                    )
                else:
                    nc.scalar.activation(eB[:], sB[:], AF.Exp, scale=SCALE)
                cur["eA"].append(eA)
                cur["eB"].append(eB)
                if debug and it == 0:
                    nc.sync.dma_start(
                        dbg["E"][0, 0, tt * 128 : (tt + 1) * 128, :],
                        eA[:],
                    )
                    nc.sync.dma_start(
                        dbg["E"][0, 1, tt * 128 : (tt + 1) * 128, :],
                        eB[:],
                    )

                if prev is not None:
                    if tt == 0:
                        emit_pv(prev, 0)
                    elif tt == 1:
                        emit_pv(prev, 1)
                    elif tt == 3:
                        emit_gather(prev, 0)
                    elif tt == 4:
                        emit_gather(prev, 1)
                    elif tt == 5:
                        emit_finish(prev, 0)
                    elif tt == 6:
                        emit_finish(prev, 1)
                else:
                    if tt == 0:
                        emit_qk(1)
                    elif tt == 2:
                        emit_qk(2)
                    elif tt == 4:
                        emit_v(range(0, 4))
                    elif tt == 6:
                        emit_v(range(4, TT))
            prev = cur

        emit_pv(prev, 0)
        emit_pv(prev, 1)
        emit_gather(prev, 0)
        emit_op(0, 0, 1)
        emit_gather(prev, 1)
        emit_op(1, 0, 1)
        emit_finish(prev, 0)
        emit_op(2, 0, 1)
        emit_finish(prev, 1)

        if debug:
            for i in range(CT):
                nc.sync.dma_start(
                    dbg["qT"][i * 128 : (i + 1) * 128, :], qT_sb[i][:].bitcast(F32)
                )
            for i in range(6):
                nc.sync.dma_start(dbg["attn"][i], attn[i][:].bitcast(F32))

        # ---------------- output projection (remaining chunks) ----------------
        ci = 0
        for ot in range(CT):
            for th in range(TH):
                if (ot, th) in op_done:
                    continue
                emit_op(ot, th, ci % 2)
                ci += 1

    nc.compile()
    return nc


def _prep_inputs(x, conv_w, bn_gamma, bn_beta, bn_mean, bn_var, wq, wk, wv, wo):
    f32 = np.float32
    inv = (bn_gamma / np.sqrt(bn_var + BN_EPS)).astype(f32)
    w9 = (conv_w.reshape(C, 9) * inv[:, None]).astype(f32)
    bias = (bn_beta - bn_mean * inv).astype(f32).reshape(C, 1)
    diag = np.zeros((3, 128, 9, 128), f32)
    for i in range(3):
        for p in range(128):
            diag[i, p, :, p] = w9[i * 128 + p, :]
    diag = diag.reshape(3, 128, 9 * 128).astype(BF16NP)
    wqT = np.ascontiguousarray(np.asarray(wq, f32).T)
    wkT = np.ascontiguousarray(np.asarray(wk, f32).T)
    wvT = np.ascontiguousarray(np.asarray(wv, f32).T)
    woT = np.asarray(wo, f32).T  # [c, o]
    # pair-tile padded wo: pair pt=2g+p holds heads (4g+2p) at rows 0-31,
    # (4g+2p+1) at rows 64-95
    woP = np.zeros((6, 128, C), f32)
    for g in range(3):
        for p in range(2):
            h_e = 4 * g + 2 * p
            woP[2 * g + p, 0:32] = woT[h_e * 32 : (h_e + 1) * 32]
            woP[2 * g + p, 64:96] = woT[(h_e + 1) * 32 : (h_e + 2) * 32]
    indg = np.zeros((128, 2), f32)
    indg[32, 0] = 1.0
    indg[96, 1] = 1.0
    bind2 = np.zeros((2, 128), f32)
    bind2[0, 0:32] = 1.0
    bind2[1, 64:96] = 1.0
    xpad = np.zeros((B, C, 34, 34), f32)
    xs = np.asarray(x, f32).transpose(0, 2, 1).reshape(B, C, 32, 32)
    xpad[:, :, 1:33, 1:33] = xs
    xpad = xpad.reshape(B, C, 34 * 34).astype(BF16NP)
    maps = []
    for b in range(B):
        maps.append(
            {
                "xpad": xpad[b],
                "diag": diag,
                "bias": bias,
                "wqT": wqT,
                "wkT": wkT,
                "wvT": wvT,
                "woP": woP,
                "indg": indg,
                "bind2": bind2,
            }
        )
    return maps


def kernel(x, conv_w, bn_gamma, bn_beta, bn_mean, bn_var, wq, wk, wv, wo, h, w,
           **kw):
    assert int(h) == HH and int(w) == WW
    from concourse.bass_utils import run_bass_kernel_spmd

    if "nc" not in _CACHE:
        _CACHE["nc"] = _build()
    nc = _CACHE["nc"]
    maps = _prep_inputs(
        x, conv_w, bn_gamma, bn_beta, bn_mean, bn_var, wq, wk, wv, wo
    )
    res = run_bass_kernel_spmd(nc, maps, list(range(NCORES)))
    out = np.stack([res.results[b]["outT"].T for b in range(B)])
    return out.astype(np.float32)



# revision 66
# speedup vs baseline: 1.0185x; 1.0185x over previous
"""Trainium2 Bass kernel v2: conv-projected MHA, engine-parallel softmax.

Per batch item (one NeuronCore each):
  y = BN(depthwise3x3(x))  ->  q/k/v proj  ->  attention  ->  out proj

Key performance structure vs v1:
 - softmax exp split across ACT (exact table exp) and DVE (Schraudolph
   bit-trick exp via tensor_scalar f32->int16, bitcast to bf16); logits
   span only ~[-3.1, 3.1] so the 3% approx error yields ~1e-2 final
   rel err (tolerance 2e-2).
 - scores: 4-way row-tiled (tile_position (32j,0)) K=32 matmuls into two
   [128,1024] PSUM tiles from a 3-slot ring -> PE/ACT/DVE fully pipelined.
 - PV: quad column-tiled per head pair: v (M=32) at (0,0)/(0,64) and the
   softmax-denominator ones-column (M=1) at (0,32)/(0,96), bf16.
 - renorm: gather D rows {32,96} to partitions 0/1 via indicator matmul,
   one batched reciprocal, K=2 broadcast matmul, tensor_tensor multiply.
 - PSUM: "s" tag 3x2 banks + "ov" 1 + "sgrb" 1 = 8 banks.
 - software pipeline: PV/renorm of iteration i-1 interleaved into the
   scores/exp stream of iteration i.
"""
import sys

sys.path.insert(0, "/opt/trn_rl_repo")
from contextlib import ExitStack

import numpy as np
import ml_dtypes

BF16NP = ml_dtypes.bfloat16

B, T, C = 8, 1024, 384
NH, DH = 12, 32
HH = WW = 32
SCALE = float(DH) ** 0.5
BN_EPS = 1e-5
NCORES = 8

# Schraudolph exp constants (bf16 / int16 variant), x = logits
A16 = 2.0 ** 7 / np.log(2.0)  # 184.664965...
B16 = 16250.0
A16S = A16 * SCALE  # folded logit scale (psum holds q.k without scale)

# which B-tiles (heads 2,3 of each group) go to DVE per (lh,g); rest ACT
DVE_B_TTS = (0, 1, 2, 3, 4, 5)

_CACHE = {}


def _build(debug=False):
    import concourse.bass as bass
    import concourse.tile as tile
    from concourse import bacc, mybir
    from concourse.masks import make_identity

    F32 = mybir.dt.float32
    F32R = mybir.dt.float32r
    BF16 = mybir.dt.bfloat16
    I16 = mybir.dt.int16
    AF = mybir.ActivationFunctionType
    ALU = mybir.AluOpType

    nc = bacc.Bacc("TRN2", target_bir_lowering=False, debug=False)

    xpad_d = nc.dram_tensor("xpad", [C, 34 * 34], BF16, kind="ExternalInput").ap()
    diag_d = nc.dram_tensor("diag", [3, 128, 9 * 128], BF16, kind="ExternalInput").ap()
    bias_d = nc.dram_tensor("bias", [C, 1], F32, kind="ExternalInput").ap()
    wqT_d = nc.dram_tensor("wqT", [C, C], F32R, kind="ExternalInput").ap()
    wkT_d = nc.dram_tensor("wkT", [C, C], F32R, kind="ExternalInput").ap()
    wvT_d = nc.dram_tensor("wvT", [C, C], F32R, kind="ExternalInput").ap()
    woP_d = nc.dram_tensor("woP", [6, 128, C], F32R, kind="ExternalInput").ap()
    indg_d = nc.dram_tensor("indg", [128, 2], F32R, kind="ExternalInput").ap()
    bind2_d = nc.dram_tensor("bind2", [2, 128], F32R, kind="ExternalInput").ap()
    outT_d = nc.dram_tensor("outT", [C, T], BF16, kind="ExternalOutput").ap()
    dbg = {}
    if debug:
        dbg["y"] = nc.dram_tensor("dbg_y", [C, T], F32, kind="ExternalOutput").ap()
        dbg["qT"] = nc.dram_tensor("dbg_qT", [C, T], F32, kind="ExternalOutput").ap()
        dbg["E"] = nc.dram_tensor(
            "dbg_E", [2, 2, T, 1024], BF16, kind="ExternalOutput"
        ).ap()  # (lh=0,g=0) A/B tiles
        dbg["ovs"] = nc.dram_tensor(
            "dbg_ovs", [2, 128, 512], F32, kind="ExternalOutput"
        ).ap()  # (lh0,g0) pairs
        dbg["attn"] = nc.dram_tensor(
            "dbg_attn", [6, 128, T], F32, kind="ExternalOutput"
        ).ap()

    CT = C // 128  # 3 c-tiles / head groups
    TT = T // 128  # 8 t-tiles
    TH = T // 512  # 2 l halves

    with tile.TileContext(nc) as tc, ExitStack() as top:
        persist = top.enter_context(tc.tile_pool(name="persist", bufs=1))
        sps = top.enter_context(tc.tile_pool(name="sps", bufs=3, space="PSUM"))
        auxps = top.enter_context(tc.tile_pool(name="auxps", bufs=1, space="PSUM"))
        epool = top.enter_context(tc.tile_pool(name="epool", bufs=26))
        ovspool = top.enter_context(tc.tile_pool(name="ovspool", bufs=3))
        rrpool = top.enter_context(tc.tile_pool(name="rrpool", bufs=2))
        ocpool = top.enter_context(tc.tile_pool(name="ocpool", bufs=4))

        y_sb = [persist.tile([128, T], F32R, tag=f"y{i}", name=f"y{i}") for i in range(CT)]
        qT_sb = [persist.tile([128, T], F32R, tag=f"q{i}", name=f"q{i}") for i in range(CT)]
        kT_sb = [persist.tile([128, T], F32R, tag=f"k{i}", name=f"k{i}") for i in range(CT)]
        vaug = [persist.tile([128, NH, 34], BF16, tag=f"va{i}", name=f"va{i}") for i in range(TT)]
        attn = [persist.tile([128, T], F32R, tag=f"at{i}", name=f"at{i}") for i in range(6)]
        bias_sb = [persist.tile([128, 1], F32, tag=f"b{i}", name=f"b{i}") for i in range(CT)]
        indg_sb = persist.tile([128, 2], F32R, tag="indg")
        bind2_sb = persist.tile([2, 128], F32R, tag="bind2")
        wT_sb = {}
        woP_sb = [persist.tile([128, C], F32R, tag=f"woP{i}", name=f"woP{i}") for i in range(6)]

        # ---------------- prefix: conv -> y ----------------
        with ExitStack() as ph1:
            convpool = ph1.enter_context(tc.tile_pool(name="convpool", bufs=1))
            xp = [convpool.tile([128, 34 * 34], BF16, tag=f"xp{i}", name=f"xp{i}") for i in range(CT)]
            diag = [convpool.tile([128, 9, 128], BF16, tag=f"dg{i}", name=f"dg{i}") for i in range(CT)]

            # PE p-state warmup on a dedicated uninitialized SBUF tile
            # (no input deps): burns the clock-ramp window before inputs land
            warm_sb = convpool.tile([128, 512], BF16, tag="warm")
            nc.vector.memset(warm_sb[:].bitcast(F32), 1.0)
            for tt in range(TT):
                nc.vector.memset(vaug[tt][:, :, 32:34].bitcast(F32), 0.0)
                nc.vector.memset(vaug[tt][:, :, 32:33], 1.0)
            for wi in range(10):
                wp = sps.tile([128, 512], F32, tag="s", name="warm")
                nc.tensor.matmul(
                    wp[:], warm_sb[:, 0:128], warm_sb[:],
                    start=True, stop=True, skip_group_check=True,
                )
            for nm in ("q", "k", "v"):
                wT_sb[nm] = [persist.tile([128, C], F32R, tag=f"w{nm}{i}", name=f"w{nm}{i}") for i in range(CT)]
            for i in (0, 1):
                nc.sync.dma_start(xp[i][:], xpad_d[i * 128 : (i + 1) * 128, :])
                nc.sync.dma_start(
                    diag[i][:].rearrange("p a b -> p (a b)"), diag_d[i]
                )
                nc.sync.dma_start(bias_sb[i][:], bias_d[i * 128 : (i + 1) * 128, :])
            for nm, d in (("q", wqT_d), ("k", wkT_d)):
                for i in range(CT):
                    nc.sync.dma_start(wT_sb[nm][i][:], d[i * 128 : (i + 1) * 128, :])
            nc.sync.dma_start(xp[2][:], xpad_d[256:384, :])
            nc.sync.dma_start(diag[2][:].rearrange("p a b -> p (a b)"), diag_d[2])
            nc.sync.dma_start(bias_sb[2][:], bias_d[256:384, :])
            for i in range(CT):
                nc.sync.dma_start(wT_sb["v"][i][:], wvT_d[i * 128 : (i + 1) * 128, :])
            for i in range(6):
                nc.sync.dma_start(woP_sb[i][:], woP_d[i])
            nc.sync.dma_start(indg_sb[:], indg_d)
            nc.sync.dma_start(bind2_sb[:], bind2_d)

            # conv: 9 accumulating diag matmuls per (c-tile, t-half)
            for i in range(CT):
                for th in range(TH):
                    yp = sps.tile([128, 512], F32, tag="s", name="convps")
                    r0 = th * 16
                    for k in range(9):
                        dy, dx = k // 3 - 1, k % 3 - 1
                        off = (r0 + 1 + dy) * 34 + (1 + dx)
                        rhs = bass.AP(
                            tensor=xp[i].tensor,
                            offset=xp[i].offset + off,
                            ap=[list(p) for p in xp[i].ap[:1]] + [[34, 16], [1, 32]],
                        )
                        nc.tensor.matmul(
                            yp[:].rearrange("p (a b) -> p a b", a=16),
                            diag[i][:, k, :],
                            rhs,
                            start=(k == 0),
                            stop=(k == 8),
                        )
                    nc.vector.tensor_scalar_add(
                        y_sb[i][:, th * 512 : (th + 1) * 512], yp[:], bias_sb[i][:]
                    )
            if debug:
                for i in range(CT):
                    nc.sync.dma_start(
                        dbg["y"][i * 128 : (i + 1) * 128, :], y_sb[i][:].bitcast(F32)
                    )

        # ---------------- prefix: q/k group 0 ----------------
        def emit_qk(ot):
            for nm, dst in (("q", qT_sb), ("k", kT_sb)):
                for th in range(TH):
                    pp = sps.tile([128, 512], F32, tag="s", name="qkps")
                    for kt in range(CT):
                        nc.tensor.matmul(
                            pp[:],
                            wT_sb[nm][kt][:, ot * 128 : (ot + 1) * 128],
                            y_sb[kt][:, th * 512 : (th + 1) * 512],
                            start=(kt == 0),
                            stop=(kt == CT - 1),
                        )
                    nc.scalar.activation(
                        dst[ot][:, th * 512 : (th + 1) * 512], pp[:], AF.Copy
                    )

        def emit_v(tts):
            for tt in tts:
                vp = sps.tile([128, C], F32, tag="s", name="vps")
                for kt in range(CT):
                    nc.tensor.matmul(
                        vp[:],
                        y_sb[kt][:, tt * 128 : (tt + 1) * 128],
                        wT_sb["v"][kt][:],
                        start=(kt == 0),
                        stop=(kt == CT - 1),
                    )
                nc.vector.tensor_copy(
                    vaug[tt][:, :, 0:32], vp[:].rearrange("p (h d) -> p h d", h=NH)
                )

        emit_qk(0)
        if debug:
            pass  # qT dumped at end

        # ---------------- attention ----------------
        LHG = [(lh, g) for lh in range(TH) for g in range(CT)]

        def emit_pv(st, p):
            g = st["g"]
            ov = auxps.tile([128, 512], F32, tag="ov", name="ov", bufs=2)
            E = st["eA"] if p == 0 else st["eB"]
            h_e = 4 * g + 2 * p
            h_o = h_e + 1
            for tt in range(TT):
                s0, s1 = (tt == 0), (tt == TT - 1)
                nc.tensor.matmul(
                    ov[0:33, :], vaug[tt][:, h_e, 0:33], E[tt][:, 0:512],
                    start=s0, stop=s1, tile_position=(0, 0),
                )
                nc.tensor.matmul(
                    ov[64:97, :], vaug[tt][:, h_o, 0:33], E[tt][:, 512:1024],
                    start=s0, stop=s1, tile_position=(0, 64),
                )
            st["ov"][p] = ov

        def emit_gather(st, p):
            ovs = ovspool.tile([128, 512], F32R, tag="ovs", name="ovs")
            nc.vector.tensor_copy(ovs[:], st["ov"][p][:])
            st["ovs"][p] = ovs
            sg = sps.tile([4, 512], F32, tag="s", name="sg")
            nc.tensor.matmul(
                sg[0:2, :], indg_sb[:], st["ovs"][p][:], start=True, stop=True
            )
            st["sg"][p] = sg

        def emit_finish(st, p):
            lh, g = st["lh"], st["g"]
            rrf = rrpool.tile([2, 512], F32, tag="rrf", name="rrf")
            nc.vector.reciprocal_approx_fast(rrf[0:2, :], st["sg"][p][0:2, :])
            rr = rrpool.tile([2, 512], F32R, tag="rr", name="rr")
            nc.scalar.activation(rr[0:2, :], rrf[0:2, :], AF.Copy)
            Rb = sps.tile([128, 512], F32, tag="s", name="Rb")
            nc.tensor.matmul(
                Rb[:], bind2_sb[:], rr[0:2, :], start=True, stop=True
            )
            at = attn[2 * g + p]
            nc.vector.tensor_tensor(
                at[:, lh * 512 : (lh + 1) * 512], st["ovs"][p][:].bitcast(F32),
                Rb[:], ALU.mult,
            )
            if debug and lh == 0 and g == 0:
                nc.sync.dma_start(dbg["ovs"][p], st["ovs"][p][:].bitcast(F32))

        op_done = set()

        def emit_op(ot, th, ceng):
            op = sps.tile([128, 512], F32, tag="s", name="op")
            for pt in range(6):
                nc.tensor.matmul(
                    op[:],
                    woP_sb[pt][:, ot * 128 : (ot + 1) * 128],
                    attn[pt][:, th * 512 : (th + 1) * 512],
                    start=(pt == 0),
                    stop=(pt == 5),
                )
            oc = ocpool.tile([128, 512], BF16, tag="oc", name="oc")
            nc.scalar.activation(oc[:], op[:], AF.Copy)
            nc.sync.dma_start(
                outT_d[ot * 128 : (ot + 1) * 128, th * 512 : (th + 1) * 512],
                oc[:],
            )
            op_done.add((ot, th))

        prev = None
        for it, (lh, g) in enumerate(LHG):
            cur = {
                "lh": lh, "g": g, "eA": [], "eB": [],
                "ov": [None, None], "ovs": [None, None], "sg": [None, None],
            }
            for tt in range(TT):
                sA = sps.tile([128, 1024], F32, tag="s", name="sA")
                sB = sps.tile([128, 1024], F32, tag="s", name="sB")
                for j, dst in (
                    (0, sA[:, 0:512]),
                    (1, sA[:, 512:1024]),
                    (2, sB[:, 0:512]),
                    (3, sB[:, 512:1024]),
                ):
                    nc.tensor.matmul(
                        dst,
                        kT_sb[g][32 * j : 32 * (j + 1), tt * 128 : (tt + 1) * 128],
                        qT_sb[g][32 * j : 32 * (j + 1), lh * 512 : (lh + 1) * 512],
                        start=True,
                        stop=True,
                        tile_position=(32 * j, 0),
                    )
                eA = epool.tile([128, 1024], BF16, tag="E", name="eA")
                eB = epool.tile([128, 1024], BF16, tag="E", name="eB")
                nc.scalar.activation(eA[:], sA[:], AF.Exp, scale=SCALE)
                if tt in DVE_B_TTS or prev is None:
                    nc.vector.tensor_scalar(
                        eB[:].bitcast(I16), sB[:], float(A16S), float(B16),
                        ALU.mult, ALU.add,
                    )
                else:
                    nc.scalar.activation(eB[:], sB[:], AF.Exp, scale=SCALE)
                cur["eA"].append(eA)
                cur["eB"].append(eB)
                if debug and it == 0:
                    nc.sync.dma_start(
                        dbg["E"][0, 0, tt * 128 : (tt + 1) * 128, :],
                        eA[:],
                    )
                    nc.sync.dma_start(
                        dbg["E"][0, 1, tt * 128 : (tt + 1) * 128, :],
                        eB[:],
                    )

                if prev is not None:
                    if tt == 0:
                        emit_pv(prev, 0)
                    elif tt == 1:
                        emit_pv(prev, 1)
                    elif tt == 3:
                        emit_gather(prev, 0)
                    elif tt == 4:
                        emit_gather(prev, 1)
                    elif tt == 5:
                        emit_finish(prev, 0)
                    elif tt == 6:
                        emit_finish(prev, 1)
                else:
                    if tt == 0:
                        emit_qk(1)
                    elif tt == 2:
                        emit_qk(2)
                    elif tt == 4:
                        emit_v(range(0, 4))
                    elif tt == 6:
                        emit_v(range(4, TT))
            prev = cur

        emit_pv(prev, 0)
        emit_pv(prev, 1)
        emit_gather(prev, 0)
        emit_op(0, 0, 1)
        emit_gather(prev, 1)
        emit_op(1, 0, 1)
        emit_finish(prev, 0)
        emit_op(2, 0, 1)
        emit_finish(prev, 1)

        if debug:
            for i in range(CT):
                nc.sync.dma_start(
                    dbg["qT"][i * 128 : (i + 1) * 128, :], qT_sb[i][:].bitcast(F32)
                )
            for i in range(6):
                nc.sync.dma_start(dbg["attn"][i], attn[i][:].bitcast(F32))

        # ---------------- output projection (remaining chunks) ----------------
        ci = 0
        for ot in range(CT):
            for th in range(TH):
                if (ot, th) in op_done:
                    continue
                emit_op(ot, th, ci % 2)
                ci += 1

    nc.compile()
    return nc


def _prep_inputs(x, conv_w, bn_gamma, bn_beta, bn_mean, bn_var, wq, wk, wv, wo):
    f32 = np.float32
    inv = (bn_gamma / np.sqrt(bn_var + BN_EPS)).astype(f32)
    w9 = (conv_w.reshape(C, 9) * inv[:, None]).astype(f32)
    bias = (bn_beta - bn_mean * inv).astype(f32).reshape(C, 1)
    diag = np.zeros((3, 128, 9, 128), f32)
    for i in range(3):
        for p in range(128):
            diag[i, p, :, p] = w9[i * 128 + p, :]
    diag = diag.reshape(3, 128, 9 * 128).astype(BF16NP)
    wqT = np.ascontiguousarray(np.asarray(wq, f32).T)
    wkT = np.ascontiguousarray(np.asarray(wk, f32).T)
    wvT = np.ascontiguousarray(np.asarray(wv, f32).T)
    woT = np.asarray(wo, f32).T  # [c, o]
    # pair-tile padded wo: pair pt=2g+p holds heads (4g+2p) at rows 0-31,
    # (4g+2p+1) at rows 64-95
    woP = np.zeros((6, 128, C), f32)
    for g in range(3):
        for p in range(2):
            h_e = 4 * g + 2 * p
            woP[2 * g + p, 0:32] = woT[h_e * 32 : (h_e + 1) * 32]
            woP[2 * g + p, 64:96] = woT[(h_e + 1) * 32 : (h_e + 2) * 32]
    indg = np.zeros((128, 2), f32)
    indg[32, 0] = 1.0
    indg[96, 1] = 1.0
    bind2 = np.zeros((2, 128), f32)
    bind2[0, 0:32] = 1.0
    bind2[1, 64:96] = 1.0
    xpad = np.zeros((B, C, 34, 34), f32)
    xs = np.asarray(x, f32).transpose(0, 2, 1).reshape(B, C, 32, 32)
    xpad[:, :, 1:33, 1:33] = xs
    xpad = xpad.reshape(B, C, 34 * 34).astype(BF16NP)
    maps = []
    for b in range(B):
        maps.append(
            {
                "xpad": xpad[b],
                "diag": diag,
                "bias": bias,
                "wqT": wqT,
                "wkT": wkT,
                "wvT": wvT,
                "woP": woP,
                "indg": indg,
                "bind2": bind2,
            }
        )
    return maps


def kernel(x, conv_w, bn_gamma, bn_beta, bn_mean, bn_var, wq, wk, wv, wo, h, w,
           **kw):
    assert int(h) == HH and int(w) == WW
    from concourse.bass_utils import run_bass_kernel_spmd

    if "nc" not in _CACHE:
        _CACHE["nc"] = _build()
    nc = _CACHE["nc"]
    maps = _prep_inputs(
        x, conv_w, bn_gamma, bn_beta, bn_mean, bn_var, wq, wk, wv, wo
    )
    res = run_bass_kernel_spmd(nc, maps, list(range(NCORES)))
    out = np.stack([res.results[b]["outT"].T for b in range(B)])
    return out.astype(np.float32)

